# revision 4
# baseline (speedup 1.0000x reference)
"""MoE conv block via 1D-Winograd F(2,3) on 8 Trainium2 NeuronCores.

Strategy (v3; direct-conv baseline was 271us, this measures ~188us):
  - Router (global-avg-pool -> gate -> softmax -> top-2 -> renorm) is tiny
    and runs on host; conv is linear in weights so the top-2 expert convs
    collapse into ONE conv per sample with combined weights.
  - Direct 3x3 conv is PE-bound at the bf16 matmul roofline (9 MACs/out,
    246us of matmul rows/core). 1D Winograd F(2,3) along the row axis cuts
    that to 6 MACs/out (164us of rows):
        t0 = x[2q]   - x[2q+2]        (input transform, DVE tensor_tensor,
        t1 = x[2q+1] + x[2q+2]         2x bf16 fast path; plain TT only --
        t2 = x[2q+2] - x[2q+1]         scalar_tensor_tensor never hits it)
        t3 = x[2q+1] - x[2q+3]
        M_r[co,q,x] = sum_s sum_ci Gw[r,s][co,ci] t_r[ci,q,x+s]   (PE)
        y[2q]   = M_0 + M_1 + M_2 + bias      (inverse transform)
        y[2q+1] = M_1 - M_2 - M_3 + bias
    with Gw[r,s] = sum_dy G[r,dy] W[:,:,dy,s] precombined on host.
  - Data-parallel over batch: 4 samples/core x 8 cores.
  - Per (sample, quarter, co-half): 4 PSUM banks (one per plane), 24
    accumulated matmuls (2 ci x 4 r x 3 s) of N=512; plane order (1,2,0,3).
    ACT evicts ALL four planes to bf16 SBUF (a=m1+bias, copies of m0/m2/m3)
    so PSUM recycling never waits on the DVE queue and all four DVE
    inverse-transform ops run in the 2x bf16 SBUF fast path.
  - x is zero-padded bf16 on host, streamed as [128,18,66] quarter tiles on
    the sync ring; per-DMA latency is ~4.7us so each quarter feeds BOTH
    co-half passes back-to-back (coh-inner group order) to outpace DMA.
    The DVE queue is strictly in-order, so input transforms are issued
    just-in-time, 2 groups ahead of consumption -- an IT op waiting on its
    x DMA would otherwise block queued OT work and stall PSUM recycling.
  - Weights ride the pool (gpsimd) DMA queue: pool DGE issues never block
    the engine, unlike scalar/sync whose dma_start stalls with queued
    transfers. First-needed tile leads, then bias, then the rest.
  - 26 warmup matmuls bridge the PE clock ramp until the first x quarter +
    IT complete (~12.4us); keeping the PE gapless avoids p-state resets.
  - Tail: the last group runs as two half-row chains so the exposed
    post-matmul OT+store chain is half-sized; all trailing stores ride
    sync/scalar HWDGE -- pool SWDGE generation + drain (~4us) would gate
    the exit barrier. Output is bf16 (host upcasts; total rel err ~6.6e-3
    vs the 2e-2 budget).
"""

import time

import numpy as np
import ml_dtypes

B, C, H, W = 32, 256, 64, 64
E, TOP_K = 8, 2
N_CORES = 8
BPC = B // N_CORES  # samples per core
HP = H + 2          # padded spatial extent (rows -1..64, cols -1..64)
NWARM = 26          # PE warmup matmuls (clock ramp; bridges to first real mm)

_COMPILED = None


def _build():
    global _COMPILED
    if _COMPILED is not None:
        return _COMPILED

    import concourse.bacc as bacc
    import concourse.tile as tile
    from concourse import mybir

    f32 = mybir.dt.float32
    bf16 = mybir.dt.bfloat16
    ADD = mybir.AluOpType.add
    SUB = mybir.AluOpType.subtract
    MULT = mybir.AluOpType.mult

    nc = bacc.Bacc("TRN2", target_bir_lowering=False, debug=False)
    # x pre-padded + bf16 on host: [b, ci, 66, 66]
    x_d = nc.dram_tensor("x", [BPC, C, HP, HP], bf16, kind="ExternalInput").ap()
    # Winograd weights [b, co_half, ci, plane(r*3+s), co%128] bf16
    w_d = nc.dram_tensor("w", [BPC, 2, C, 12, 128], bf16, kind="ExternalInput").ap()
    # bias [c%128, co_half, sample] f32
    b_d = nc.dram_tensor("bias", [128, 2, BPC], f32, kind="ExternalInput").ap()
    # bf16 output; host upcasts after gather
    o_d = nc.dram_tensor("out", [BPC, C, H, W], bf16, kind="ExternalOutput").ap()

    with tile.TileContext(nc) as tc:
        with (
            tc.tile_pool(name="warmp", bufs=1) as warm_pool,
            tc.tile_pool(name="xp", bufs=8) as x_pool,
            tc.tile_pool(name="tp", bufs=2) as t_pool,
            tc.tile_pool(name="wtp", bufs=1) as wt_pool,
            tc.tile_pool(name="biasp", bufs=1) as bias_pool,
            tc.tile_pool(name="up", bufs=2) as u_pool,
            tc.tile_pool(name="otp", bufs=4) as ot_pool,
            tc.tile_pool(name="psump", bufs=8, space="PSUM") as psum_pool,
        ):
            # weights all prefetched: wts[b][coh][ch]: [128, 12, 128]
            wts = [[[wt_pool.tile([128, 12, 128], bf16, name=f"w{b}{coh}{ch}")
                     for ch in range(2)] for coh in range(2)] for b in range(BPC)]
            bt = bias_pool.tile([128, 2, BPC], f32, name="bt")
            wz = warm_pool.tile([128, 256], bf16, name="wz")

            # gpsimd: warmup stationary first (PE clock ramp)
            nc.gpsimd.memset(wz[:], 0.0)

            # x quarter-row tiles [128, 18, 66]: quarter qt covers padded
            # rows 16qt..16qt+17 (tile q-rows 8qt..8qt+7, 2-row overlap)
            xts = {}

            def xdma(b, ch, qt, ring=None):
                t = x_pool.tile([128, 18, HP], bf16, name="xt")
                xts[(b, ch, qt)] = t
                (ring or nc.sync).dma_start(
                    t[:], x_d[b, ch * 128:(ch + 1) * 128, 16 * qt:16 * qt + 18])

            def wdma(b, coh, ch):
                # pool DGE issues never block the engine (unlike scalar/sync)
                nc.gpsimd.dma_start(wts[b][coh][ch][:],
                                    w_d[b, coh, ch * 128:(ch + 1) * 128])

            # loads: x on sync ring in consumption order. The very first
            # ch1 piece rides the scalar queue as its ONLY early DMA (a
            # single issue doesn't hit the credit-blocking that multiple
            # scalar DMAs do), arriving in parallel with sync's piece 1 so
            # the first group's ch1 phase never waits.
            xdma(0, 1, 0, ring=nc.scalar)
            for b in range(BPC):
                for qt in range(4):
                    for ch in range(2):
                        if b == 0 and ch == 1 and qt == 0:
                            continue
                        xdma(b, ch, qt)
            # first-needed w tiles lead the pool queue (w2 ahead of bias:
            # it gates the first group's ch1 phase), then the rest
            wdma(0, 0, 0)
            wdma(0, 0, 1)
            nc.gpsimd.dma_start(bt[:], b_d[:])
            wdma(0, 1, 0)
            wdma(0, 1, 1)
            for b in range(1, BPC):
                for coh in range(2):
                    for ch in range(2):
                        wdma(b, coh, ch)

            # --- PE warmup
            wps = psum_pool.tile([128, 8, W], f32, name="ps")
            for i in range(NWARM):
                nc.tensor.matmul(wps[:, 0:4, :], wz[:, 0:128], wz[:],
                                 start=(i == 0), stop=(i == NWARM - 1))

            # input transform for (b, ch, h) -> t planes [128, 32, 66] bf16
            tts = {}

            def it_tiles(b, qt):
                # one tile per (r, ch, qt), exactly one GEMM row-block each,
                # written by exactly ONE IT op (fine-grained pipeline)
                for r in range(4):
                    for ch in range(2):
                        tts[(b, r, ch, qt)] = t_pool.tile(
                            [128, 8, HP], bf16, name=f"t{r}{ch}{qt}")

            def it_ops(b, ch, qt):
                xs = xts[(b, ch, qt)]
                r0 = xs[:, 0:16:2, :]
                r1 = xs[:, 1:17:2, :]
                r2 = xs[:, 2:18:2, :]
                r3 = xs[:, 3:18:2, :]
                dst = [tts[(b, r, ch, qt)][:] for r in range(4)]
                # plain tensor_tensor: hits the DVE 2x bf16 fast path
                # (scalar_tensor_tensor never does, measured on HW)
                nc.vector.tensor_sub(dst[0], r0, r2)
                nc.vector.tensor_add(dst[1], r1, r2)
                nc.vector.tensor_sub(dst[2], r2, r1)
                nc.vector.tensor_sub(dst[3], r1, r3)

            def chain(ps, b, coh, g, ch, plane_order):
                for r in plane_order:
                    for s in range(3):
                        nc.tensor.matmul(
                            ps[r][:],
                            wts[b][coh][ch][:, 3 * r + s, :],
                            tts[(b, r, ch, g)][:, :, s:s + W],
                            start=(ch == 0 and s == 0),
                            stop=(ch == 1 and s == 2))

            def gemm_group(b, coh, g, plane_order=(1, 2, 0, 3)):
                """24 matmuls -> 4 PSUM planes for out rows 16g..16g+15."""
                ps = [psum_pool.tile([128, 8, W], f32, name="ps")
                      for r in range(4)]
                for ch in range(2):
                    chain(ps, b, coh, g, ch, plane_order)
                return ps

            def ot_ops(b, coh, g, ps, split_store=False):
                """Inverse transform + bias -> bf16 out tile, store.

                DVE/ACT ops may read at most ONE PSUM operand each.
                ACT evicts m1 (+bias) and m2 so PSUM banks free early and
                DVE only gates on m0/m3:
                  a  = m1 + bias (ACT)   a2 = m2 (ACT copy)
                  u0 = a + m0    y0 = u0 + a2   (DVE, even rows)
                  v  = a - a2    y1 = v  - m3   (DVE, odd rows)
                """
                bias_ap = bt[:, coh, b:b + 1]
                ot = ot_pool.tile([128, 16, W], bf16, name="ot")
                a = u_pool.tile([128, 8, W], bf16, name="a")
                a2 = u_pool.tile([128, 8, W], bf16, name="a2")
                a0 = u_pool.tile([128, 8, W], bf16, name="a0")
                a3 = u_pool.tile([128, 8, W], bf16, name="a3")
                u0 = u_pool.tile([128, 8, W], bf16, name="u")
                v = u_pool.tile([128, 8, W], bf16, name="v")
                # ACT evicts ALL planes (in completion order 1,2,0,3): PSUM
                # recycling never waits on the DVE queue, and every DVE op
                # runs in the 2x bf16 SBUF fast path
                nc.scalar.add(a[:], ps[1][:], bias_ap)
                nc.scalar.copy(a2[:], ps[2][:])
                nc.scalar.copy(a0[:], ps[0][:])
                nc.scalar.copy(a3[:], ps[3][:])
                od = o_d[b, coh * 128:(coh + 1) * 128, 16 * g:16 * g + 16, :]
                if split_store:
                    ot_e = ot_pool.tile([128, 8, W], bf16, name="ote")
                    ot_o = ot_pool.tile([128, 8, W], bf16, name="oto")
                    nc.vector.tensor_add(u0[:], a[:], a0[:])
                    nc.vector.tensor_sub(v[:], a[:], a2[:])
                    nc.vector.tensor_add(ot_e[:], u0[:], a2[:])
                    nc.sync.dma_start(od[:, 0:16:2, :], ot_e[:])
                    nc.vector.tensor_sub(ot_o[:], v[:], a3[:])
                    nc.scalar.dma_start(od[:, 1:16:2, :], ot_o[:])
                else:
                    nc.vector.tensor_add(u0[:], a[:], a0[:])
                    nc.vector.tensor_sub(v[:], a[:], a2[:])
                    nc.vector.tensor_add(ot[:, 0:16:2, :], u0[:], a2[:])
                    nc.vector.tensor_sub(ot[:, 1:16:2, :], v[:], a3[:])
                    # alternate store queues: halves the per-queue drain.
                    # The very last coh0 store rides sync instead of pool:
                    # the pool SWDGE drain (~4us) would gate the exit
                    # barrier, while the sync queue is empty by then.
                    if coh == 0 and b == BPC - 1 and g == 3:
                        ring = nc.sync
                    else:
                        ring = nc.gpsimd if coh == 0 else nc.sync
                    ring.dma_start(od, ot[:])

            # schedule: the DVE queue is strictly in-order, so IT ops
            # must be issued just-in-time (2 groups ahead of consumption) --
            # an IT op waiting on its x DMA would otherwise block the queued
            # inverse-transform ops behind it and stall PSUM recycling.
            it_tiles(0, 0)
            for ch in range(2):
                it_ops(0, ch, 0)
            # warmup PSUM slot release rides ACT (idle-ish, reads PSUM fine)
            wsink = warm_pool.tile([128, W], f32, name="wsink")
            nc.scalar.copy(wsink[:], wps[:, 0, :])

            def half_chain(ps, b, coh, g, ch, hb, plane_order):
                """6 matmuls of N=256 into rows 4hb:4hb+4 of each plane."""
                for r in plane_order:
                    for s in range(3):
                        nc.tensor.matmul(
                            ps[r][:, 4 * hb:4 * hb + 4, :],
                            wts[b][coh][ch][:, 3 * r + s, :],
                            tts[(b, r, ch, g)][:, 4 * hb:4 * hb + 4, s:s + W],
                            start=(ch == 0 and s == 0),
                            stop=(ch == 1 and s == 2))

            def ot_half(b, coh, g, ps, hb, ring):
                bias_ap = bt[:, coh, b:b + 1]
                a = u_pool.tile([128, 4, W], bf16, name=f"ha{hb}")
                a2 = u_pool.tile([128, 4, W], bf16, name=f"ha2{hb}")
                a0 = u_pool.tile([128, 4, W], bf16, name=f"ha0{hb}")
                a3 = u_pool.tile([128, 4, W], bf16, name=f"ha3{hb}")
                u0 = u_pool.tile([128, 4, W], bf16, name=f"hu{hb}")
                v = u_pool.tile([128, 4, W], bf16, name=f"hv{hb}")
                ot_e = ot_pool.tile([128, 4, W], bf16, name=f"hoe{hb}")
                ot_o = ot_pool.tile([128, 4, W], bf16, name=f"hoo{hb}")
                sl = slice(4 * hb, 4 * hb + 4)
                nc.scalar.add(a[:], ps[1][:, sl, :], bias_ap)
                nc.scalar.copy(a2[:], ps[2][:, sl, :])
                nc.scalar.copy(a0[:], ps[0][:, sl, :])
                nc.scalar.copy(a3[:], ps[3][:, sl, :])
                nc.vector.tensor_add(u0[:], a[:], a0[:])
                nc.vector.tensor_sub(v[:], a[:], a2[:])
                nc.vector.tensor_add(ot_e[:], u0[:], a2[:])
                r0 = 16 * g + 8 * hb
                od = o_d[b, coh * 128:(coh + 1) * 128, r0:r0 + 8, :]
                ring[0].dma_start(od[0:128, 0:8:2, :], ot_e[:])
                nc.vector.tensor_sub(ot_o[:], v[:], a3[:])
                ring[1].dma_start(od[0:128, 1:8:2, :], ot_o[:])

            seq = [(b, g, coh)
                   for b in range(BPC) for g in range(4) for coh in range(2)]
            for idx, (b, g, coh) in enumerate(seq):
                last = idx == len(seq) - 1
                if last:
                    ps = [psum_pool.tile([128, 8, W], f32, name="ps")
                          for r in range(4)]
                    for ch in range(2):
                        half_chain(ps, b, coh, g, ch, 0, (1, 2, 0, 3))
                    ot_half(b, coh, g, ps, 0, (nc.sync, nc.sync))
                    for ch in range(2):
                        half_chain(ps, b, coh, g, ch, 1, (1, 2, 0, 3))
                    ot_half(b, coh, g, ps, 1, (nc.sync, nc.scalar))
                    continue
                ps = gemm_group(b, coh, g)
                ot_ops(b, coh, g, ps)
                if idx + 2 < len(seq):
                    b2, g2, coh2 = seq[idx + 2]
                    if coh2 == 0:
                        it_tiles(b2, g2)
                        for ch in range(2):
                            it_ops(b2, ch, g2)
    nc.compile()
    _COMPILED = nc
    return nc


def _route_and_combine(x, gate_w, gate_b, conv_w, conv_b):
    """Host router (mirrors reference) + per-sample Winograd weights."""
    xf = np.asarray(x, dtype=np.float32)
    pooled = xf.mean(axis=(2, 3))
    logits = pooled @ np.asarray(gate_w, np.float32).T + np.asarray(gate_b, np.float32)
    z = logits - logits.max(-1, keepdims=True)
    wgt = np.exp(z)
    wgt /= wgt.sum(-1, keepdims=True)
    top_i = np.argsort(-wgt, axis=-1, kind="stable")[:, :TOP_K]
    top_w = np.take_along_axis(wgt, top_i, axis=-1)
    tz = top_w - top_w.max(-1, keepdims=True)
    tw = np.exp(tz)
    tw /= tw.sum(-1, keepdims=True)

    cw = np.asarray(conv_w, np.float32)
    cb = np.asarray(conv_b, np.float32)
    Wc = (cw[top_i[:, 0]] * tw[:, 0, None, None, None, None]
          + cw[top_i[:, 1]] * tw[:, 1, None, None, None, None])  # [B,co,ci,3,3]
    bc = cb[top_i[:, 0]] * tw[:, 0, None] + cb[top_i[:, 1]] * tw[:, 1, None]

    # Winograd row-transform of weights: Gw[b,r,s,co,ci]
    G = np.array([[1, 0, 0], [.5, .5, .5], [.5, -.5, .5], [0, 0, 1]], np.float32)
    Gw = np.einsum("rd,boids->brsoi", G, Wc)
    # device layout [b, coh, ci, plane(3r+s), co%128]
    Wd = (Gw.reshape(B, 4, 3, 2, 128, C)
          .transpose(0, 3, 5, 1, 2, 4)
          .reshape(B, 2, C, 12, 128))
    Wd = np.ascontiguousarray(Wd).astype(ml_dtypes.bfloat16)

    bd = bc.reshape(N_CORES, BPC, 2, 128).transpose(0, 3, 2, 1)  # [core,128,2,b]

    xp = np.zeros((B, C, HP, HP), dtype=ml_dtypes.bfloat16)
    xp[:, :, 1:H + 1, 1:W + 1] = xf.astype(ml_dtypes.bfloat16)
    return xp, Wd, np.ascontiguousarray(bd).astype(np.float32)


def run_sharded(inputs, trace=False, trace_cores=None):
    from concourse.bass_utils import run_bass_kernel_spmd

    xp, Wd, bc = _route_and_combine(
        inputs["x"], inputs["gate_w"], inputs["gate_b"],
        inputs["conv_w"], inputs["conv_b"],
    )
    nc = _build()
    in_maps = []
    for k in range(N_CORES):
        s = slice(k * BPC, (k + 1) * BPC)
        in_maps.append({"x": xp[s], "w": Wd[s], "bias": bc[k]})
    last_err = None
    for attempt in range(3):
        try:
            res = run_bass_kernel_spmd(
                nc, in_maps, list(range(N_CORES)),
                trace=trace, trace_cores=trace_cores,
            )
            break
        except Exception as e:  # transient NRT flakes
            last_err = e
            time.sleep(5.0)
    else:
        raise last_err
    out = np.concatenate([r["out"] for r in res.results], axis=0)
    return out.astype(np.float32), res


def kernel(x, gate_w, gate_b, conv_w, conv_b):
    out, _ = run_sharded(
        {"x": x, "gate_w": gate_w, "gate_b": gate_b,
         "conv_w": conv_w, "conv_b": conv_b}
    )
    return out
